# revision 2
# baseline (speedup 1.0000x reference)
"""BezierAlign distributed Trainium2 kernel.

Contract: kernel(input, beziers) -> [256, 256, 16, 64] f32, computed on the
8 NeuronCores. Host side only shards/routes/reassembles:
  - ROIs are routed to cores so each core's 32 ROIs live in <= 2 adjacent
    batches (sharding_hint: route ROIs by batch / shard by ROI).
  - The feature map is resharded per core as a bf16 y-paired NHWC pixel
    table (2 batches): table row t = [ch(pixel t), ch(pixel t + W)], so one
    2-row gather element covers a bin's full 2x2 bilinear patch.
All math (bezier eval, bilinear weights, index arithmetic, gather, weighted
sum, output transpose) runs on-device.

Per-core device program (SPMD, identical on all 8 cores):
  - bezier control points -> sample coords X, Y for all 32x1024 bins via
    PE matmuls against a constant basis matrix (layout [bin%128, g*32+n]).
  - bilinear indices/weights via DVE elementwise ops; gather indices are
    folded to dma_gather's 16-partition-wrapped int16 layout with identity
    -slice PE matmuls + one permuted copy.
  - per ROI: one 1024-index dma_gather pulls 2 MiB of bf16 2x2-patch rows;
    8 groups x 8 bf16 matmuls against diag(weight) matrices apply the 4
    bilinear terms and transpose [bin, ch] -> [ch, bin] into fp32 PSUM;
    DVE/Act copies assemble bf16 [256ch, 1024bin]; 2 HWDGE DMAs write the
    ROI's NCHW output (bf16; host converts back to f32).
"""

import numpy as np

B, C, H, W = 8, 256, 128, 128
N_ROIS = 256
PH, PW = 16, 64
NB = PH * PW              # 1024 bins per ROI
NCORES = 8
R = 32                    # ROIs per core
SCALE = 0.25
PIX = H * W               # 16384 pixel rows per batch
TROWS = 2 * PIX           # table rows addressable by int16 idx (32768)
PADROWS = 192             # zero rows appended (idx+1 overrun)
BUFROWS = TROWS + PADROWS

_cache = {}


def _basis_const():
    """M[k, t]: X[n, t] = sum_k ctrl_x[n, k] * M[k, t], t = i*64 + j."""
    t = np.arange(NB)
    i, j = t // PW, t % PW
    u = j.astype(np.float64) / PW
    v = i.astype(np.float64) / PH
    co = [1.0, 3.0, 3.0, 1.0]
    M = np.zeros((8, NB), np.float64)
    for k in range(4):
        bern = co[k] * u**k * (1.0 - u) ** (3 - k)
        M[k] = SCALE * bern * (1.0 - v)       # top curve
        M[4 + k] = SCALE * bern * v           # bottom curve
    # Permute columns to the on-device (g, p) layout. Bin-at-position map
    # t(p, g) = (p%16)*64 + g*8 + p//16 makes DMA engine r (= position%16)
    # walk output row r along the curve -> near-contiguous gather addresses.
    gg = np.arange(8)[:, None]
    pp = np.arange(128)[None, :]
    tcol = (pp % 16) * 64 + gg * 8 + pp // 16
    return M[:, tcol.reshape(-1)].astype(np.float32)


def _build_program(rep=1):
    """rep>1 wraps the main ROI loop in a hardware repeat loop (benchmarking
    only — output is rewritten identically each iteration)."""
    import contextlib
    import concourse.bass as bass
    import concourse.bacc as bacc
    import concourse.tile as tile
    from concourse import mybir

    f32 = mybir.dt.float32
    bf16 = mybir.dt.bfloat16
    Alu = mybir.AluOpType
    Act = mybir.ActivationFunctionType

    nc = bacc.Bacc("TRN2", target_bir_lowering=False, debug=False)
    feat = nc.dram_tensor("feat", [BUFROWS, 2 * C], bf16, kind="ExternalInput")
    bez = nc.dram_tensor("bez", [R, 17], f32, kind="ExternalInput")
    mconst = nc.dram_tensor("mconst", [8, NB], f32, kind="ExternalInput")
    eye_d = nc.dram_tensor("eye", [128, 128], f32, kind="ExternalInput")
    eyeb_d = nc.dram_tensor("eyeb", [128, 128], bf16, kind="ExternalInput")
    id32_d = nc.dram_tensor("id32", [32, 32], f32, kind="ExternalInput")
    c16k_d = nc.dram_tensor("c16k", [1, 128], f32, kind="ExternalInput")
    out_d = nc.dram_tensor("out", [R, C, PH, PW], bf16, kind="ExternalOutput")

    # gather table view: idx unit = 1 table row (512 bf16), elem = 2 rows
    gather_src = bass.AP(feat[:].tensor, 0, [[2 * C, TROWS], [1, 4 * C]])

    with tile.TileContext(nc) as tc:
        with (
            tc.tile_pool(name="const", bufs=1) as cpool,
            tc.tile_pool(name="work", bufs=1) as wpool,
        ):
            eye = cpool.tile([128, 128], f32)
            nc.sync.dma_start(eye[:], eye_d[:])
            eyeb = cpool.tile([128, 128], bf16)
            nc.sync.dma_start(eyeb[:], eyeb_d[:])
            m_sb = cpool.tile([8, NB], f32)
            nc.sync.dma_start(m_sb[:], mconst[:])
            id32 = cpool.tile([32, 32], f32)
            nc.sync.dma_start(id32[:], id32_d[:])
            c16k = cpool.tile([1, 128], f32)
            nc.sync.dma_start(c16k[:], c16k_d[:])
            bez_sb = cpool.tile([R, 17], f32)
            nc.sync.dma_start(bez_sb[:], bez[:])

            w00 = wpool.tile([128, 256], f32)
            w01 = wpool.tile([128, 256], f32)
            w10 = wpool.tile([128, 256], f32)
            w11 = wpool.tile([128, 256], f32)
            widx = wpool.tile([128, R * 64], mybir.dt.int16)

            with (
                tc.tile_pool(name="setup", bufs=1) as spool,
                tc.tile_pool(name="psetup", bufs=1, space="PSUM") as pspool,
            ):
                # --- control points -> per-bin coords ------------------
                p_sep = spool.tile([R, 17], f32)
                nc.vector.tensor_copy(p_sep[:, 0:8], bez_sb[:, 1:17:2])
                nc.vector.tensor_copy(p_sep[:, 8:16], bez_sb[:, 2:17:2])
                nc.vector.tensor_copy(p_sep[:, 16:17], bez_sb[:, 0:1])
                pt_ps = pspool.tile([8, 3 * 32], f32)
                nc.tensor.transpose(out=pt_ps[0:8, 0:32], in_=p_sep[:, 0:8],
                                    identity=id32[:])
                nc.tensor.transpose(out=pt_ps[0:8, 32:64], in_=p_sep[:, 8:16],
                                    identity=id32[:])
                nc.tensor.transpose(out=pt_ps[0:1, 64:96], in_=p_sep[:, 16:17],
                                    identity=id32[:])
                pt = spool.tile([8, 3 * 32], f32)
                nc.vector.tensor_copy(pt[:, 0:64], pt_ps[0:8, 0:64])
                nc.vector.tensor_copy(pt[0:1, 64:96], pt_ps[0:1, 64:96])

                ps_x = pspool.tile([128, 256], f32)
                ps_y = pspool.tile([128, 256], f32)
                ps_b = pspool.tile([128, 256], f32)
                for g in range(8):
                    sl = slice(g * 32, (g + 1) * 32)
                    nc.tensor.matmul(out=ps_x[:, sl], lhsT=m_sb[:, g * 128:(g + 1) * 128],
                                     rhs=pt[0:8, 0:32], start=True, stop=True)
                    nc.tensor.matmul(out=ps_y[:, sl], lhsT=m_sb[:, g * 128:(g + 1) * 128],
                                     rhs=pt[0:8, 32:64], start=True, stop=True)
                    nc.tensor.matmul(out=ps_b[:, sl], lhsT=c16k[:],
                                     rhs=pt[0:1, 64:96], start=True, stop=True)

                # --- bilinear indices + weights ------------------------
                def T(name):
                    return spool.tile([128, 256], f32, name=name)

                xs, xl, lx, hx = T("xs"), T("xl"), T("lx"), T("hx")
                ys, yl, ly, hy = T("ys"), T("yl"), T("ly"), T("hy")
                tmp, val, tv = T("tmp"), T("val"), T("tv")
                ixf = T("ixf")

                v = nc.vector
                MAGIC = 12582912.0  # 1.5 * 2**23: (x+M)-M rounds x to nearest
                v.tensor_scalar(xs[:], ps_x[:], 0.0, None, Alu.max)
                v.tensor_scalar(xl[:], xs[:], MAGIC, -MAGIC, Alu.add, Alu.add)
                v.tensor_tensor(tmp[:], xl[:], xs[:], Alu.is_gt)
                v.tensor_tensor(xl[:], xl[:], tmp[:], Alu.subtract)
                v.tensor_scalar(xl[:], xl[:], float(W - 1), None, Alu.min)
                v.tensor_tensor(lx[:], xs[:], xl[:], Alu.subtract)
                v.tensor_scalar(tmp[:], xl[:], float(W - 1), None, Alu.is_lt)
                v.tensor_tensor(lx[:], lx[:], tmp[:], Alu.mult)
                v.tensor_scalar(hx[:], lx[:], 1.0, -1.0, Alu.subtract, Alu.mult)

                v.tensor_scalar(ys[:], ps_y[:], 0.0, None, Alu.max)
                v.tensor_scalar(yl[:], ys[:], MAGIC, -MAGIC, Alu.add, Alu.add)
                v.tensor_tensor(tmp[:], yl[:], ys[:], Alu.is_gt)
                v.tensor_tensor(yl[:], yl[:], tmp[:], Alu.subtract)
                v.tensor_scalar(yl[:], yl[:], float(H - 1), None, Alu.min)
                v.tensor_tensor(ly[:], ys[:], yl[:], Alu.subtract)
                v.tensor_scalar(tmp[:], yl[:], float(H - 1), None, Alu.is_lt)
                v.tensor_tensor(ly[:], ly[:], tmp[:], Alu.mult)
                v.tensor_scalar(hy[:], ly[:], 1.0, -1.0, Alu.subtract, Alu.mult)

                v.tensor_scalar(val[:], ps_x[:], float(W), None, Alu.is_lt)
                v.tensor_scalar(tv[:], ps_x[:], -1.0, None, Alu.is_gt)
                v.tensor_tensor(val[:], val[:], tv[:], Alu.mult)
                v.tensor_scalar(tv[:], ps_y[:], float(H), None, Alu.is_lt)
                v.tensor_tensor(val[:], val[:], tv[:], Alu.mult)
                v.tensor_scalar(tv[:], ps_y[:], -1.0, None, Alu.is_gt)
                v.tensor_tensor(val[:], val[:], tv[:], Alu.mult)
                v.tensor_tensor(hy[:], hy[:], val[:], Alu.mult)
                v.tensor_tensor(ly[:], ly[:], val[:], Alu.mult)

                v.tensor_tensor(w00[:], hy[:], hx[:], Alu.mult)
                v.tensor_tensor(w01[:], hy[:], lx[:], Alu.mult)
                v.tensor_tensor(w10[:], ly[:], hx[:], Alu.mult)
                v.tensor_tensor(w11[:], ly[:], lx[:], Alu.mult)

                v.tensor_scalar(ixf[:], yl[:], float(W), None, Alu.mult)
                v.tensor_tensor(ixf[:], ixf[:], xl[:], Alu.add)
                v.tensor_tensor(ixf[:], ixf[:], ps_b[:], Alu.add)

                # --- fold idx to dma_gather wrapped layout -------------
                # widx[r, n*64 + g*8 + q] = ix[q*16+r, g*32+n]
                fold = pspool.tile([16, 2048], f32, name="fold", tag="fold")
                for q in range(8):
                    nc.tensor.matmul(out=fold[:, q * 256:(q + 1) * 256],
                                     lhsT=eye[:, q * 16:(q + 1) * 16],
                                     rhs=ixf[:], start=True, stop=True)
                dst = (widx[0:16, :]
                       .rearrange("r (n t) -> r n t", n=R)
                       .rearrange("r n (g q) -> r n g q", g=8))
                perm = fold[:].rearrange("r (q g n) -> r n g q", q=8, g=8, n=32)
                nc.vector.tensor_copy(dst, perm)
                for k in range(1, 8):
                    nc.sync.dma_start(widx[16 * k:16 * (k + 1), :], widx[0:16, :])

            # --- main ROI loop ------------------------------------------
            with (
                tc.tile_pool(name="gath", bufs=3) as gpool,
                tc.tile_pool(name="stg", bufs=3) as stpool,
                tc.tile_pool(name="diag", bufs=3) as dpool,
                tc.tile_pool(name="pmain", bufs=8, space="PSUM") as ppool,
                tc.For_i(0, rep, 1) if rep > 1 else contextlib.nullcontext(),
            ):
                for n in range(R):
                    ga = gpool.tile([128, 8 * 1024], bf16, name="ga")
                    nc.gpsimd.dma_gather(
                        out_ap=ga[:].rearrange("p (t e) -> p t e", e=1024),
                        in_ap=gather_src,
                        idxs_ap=widx[:, n * 64:(n + 1) * 64],
                        num_idxs=1024,
                        num_idxs_reg=1024,
                        elem_size=1024,
                        elem_step=512,
                    )
                    stage = stpool.tile([128, 2048], bf16, name="stage")
                    for g in range(8):
                        col = g * 32 + n
                        d00 = dpool.tile([128, 128], bf16, name="d00")
                        d01 = dpool.tile([128, 128], bf16, name="d01")
                        d10 = dpool.tile([128, 128], bf16, name="d10")
                        d11 = dpool.tile([128, 128], bf16, name="d11")
                        nc.vector.tensor_scalar(d00[:], eyeb[:], w00[:, col:col + 1],
                                                None, Alu.mult)
                        nc.scalar.activation(d01[:], eyeb[:], Act.Copy,
                                             scale=w01[:, col:col + 1])
                        nc.vector.tensor_scalar(d10[:], eyeb[:], w10[:, col:col + 1],
                                                None, Alu.mult)
                        nc.scalar.activation(d11[:], eyeb[:], Act.Copy,
                                             scale=w11[:, col:col + 1])
                        po = ppool.tile([128, 256], f32, name="po")
                        for h in range(2):
                            osl = slice(h * 128, (h + 1) * 128)
                            base = g * 1024 + h * 128
                            nc.tensor.matmul(out=po[:, osl], rhs=d00[:],
                                             lhsT=ga[:, base:base + 128],
                                             start=True, stop=False)
                            nc.tensor.matmul(out=po[:, osl], rhs=d10[:],
                                             lhsT=ga[:, base + 256:base + 384],
                                             start=False, stop=False)
                            nc.tensor.matmul(out=po[:, osl], rhs=d01[:],
                                             lhsT=ga[:, base + 512:base + 640],
                                             start=False, stop=False)
                            nc.tensor.matmul(out=po[:, osl], rhs=d11[:],
                                             lhsT=ga[:, base + 768:base + 896],
                                             start=False, stop=True)
                        # scatter psum col p -> stage bin t(p,g); iterate
                        # (a = p//16 outer, r = p%16 inner) on both sides
                        for h in range(2):
                            src = (po[:, h * 128:(h + 1) * 128]
                                   .rearrange("c (a r) -> c a r", a=8))
                            dst = (stage[:, h * 1024:(h + 1) * 1024]
                                   .rearrange("c (r g2 a) -> c g2 a r",
                                              r=16, g2=8, a=8)[:, g, :, :])
                            if h == 0:
                                nc.vector.tensor_copy(dst, src)
                            else:
                                nc.pool.tensor_copy(dst, src)
                    for h in range(2):
                        nc.sync.dma_start(out_d[n, h * 128:(h + 1) * 128, :, :],
                                          stage[:, h * 1024:(h + 1) * 1024])

    nc.compile()
    return nc


def _route(batch):
    """Assign ROIs to cores: sorted by batch, each core spans <=2 adjacent
    batches, <=R ROIs. Returns (ids_per_core, base_per_core)."""
    order = np.argsort(batch, kind="stable")
    n = len(order)
    ids, bases = [], []
    i = 0
    for _ in range(NCORES):
        if i >= n:
            ids.append([])
            bases.append(0)
            continue
        base = int(batch[order[i]])
        cur = []
        while i < n and len(cur) < R and int(batch[order[i]]) <= base + 1:
            cur.append(int(order[i]))
            i += 1
        ids.append(cur)
        bases.append(base)
    if i < n:
        raise RuntimeError("ROI->core routing failed (batch distribution too "
                           "skewed for 8 cores x 2 batches)")
    return ids, bases


def kernel(input, beziers):
    import ml_dtypes
    from concourse.bass_utils import run_bass_kernel_spmd

    bf16 = ml_dtypes.bfloat16
    input = np.asarray(input, dtype=np.float32)
    beziers = np.asarray(beziers, dtype=np.float32)

    if "nc" not in _cache:
        _cache["nc"] = _build_program()
    nc = _cache["nc"]

    batch = beziers[:, 0].astype(np.int32)
    ids, bases = _route(batch)

    nhwc = np.ascontiguousarray(
        input.transpose(0, 2, 3, 1)).reshape(B * PIX, C).astype(bf16)
    consts = {
        "mconst": _basis_const(),
        "eye": np.eye(128, dtype=np.float32),
        "eyeb": np.eye(128, dtype=np.float32).astype(bf16),
        "id32": np.eye(32, dtype=np.float32),
        "c16k": np.full((1, 128), float(PIX), np.float32),
    }

    in_maps = []
    for c in range(NCORES):
        buf = np.zeros((BUFROWS, 2 * C), bf16)
        lo = bases[c] * PIX
        hi = min((bases[c] + 2) * PIX, B * PIX)
        nrow = hi - lo
        buf[:nrow, :C] = nhwc[lo:hi]
        # second half of row t = pixel t + W (next image row, same layout)
        buf[:nrow - W, C:] = nhwc[lo + W:hi]
        if hi < B * PIX:
            buf[nrow - W:nrow, C:] = nhwc[hi:hi + W]
        bz = np.zeros((R, 17), np.float32)
        if ids[c]:
            rows = beziers[ids[c]].copy()
            rows[:, 0] = batch[ids[c]] - bases[c]
            bz[:len(ids[c])] = rows
        in_maps.append({"feat": buf, "bez": bz, **consts})

    _cache["in_maps"] = in_maps
    res = run_bass_kernel_spmd(nc, in_maps, list(range(NCORES)))

    out = np.zeros((N_ROIS, C, PH, PW), np.float32)
    for c in range(NCORES):
        if ids[c]:
            out[ids[c]] = res.results[c]["out"][:len(ids[c])].astype(np.float32)
    return out


# revision 7
# speedup vs baseline: 1.4727x; 1.4727x over previous
"""BezierAlign distributed Trainium2 kernel.

Contract: kernel(input, beziers) -> [256, 256, 16, 64] f32, computed on the
8 NeuronCores. Host side only shards/routes/reassembles:
  - ROIs are routed to cores so each core's 32 ROIs live in <= 2 adjacent
    batches (sharding_hint: route ROIs by batch / shard by ROI).
  - The feature map is resharded per core as a bf16 y-paired NHWC pixel
    table (2 batches): table row t = [ch(pixel t), ch(pixel t + W)], so one
    2-row gather element covers a bin's full 2x2 bilinear patch.
All math (bezier eval, bilinear weights, index arithmetic, gather, weighted
sum, output transpose) runs on-device.

Per-core device program (SPMD, identical on all 8 cores):
  - bezier control points -> sample coords X, Y for all 32x1024 bins via
    PE matmuls against a constant basis matrix (layout [bin%128, g*32+n]).
  - bilinear indices/weights via DVE elementwise ops; gather indices are
    folded to dma_gather's 16-partition-wrapped int16 layout with identity
    -slice PE matmuls + one permuted copy.
  - per ROI: one 1024-index dma_gather pulls 2 MiB of bf16 2x2-patch rows;
    8 groups x 8 bf16 matmuls against diag(weight) matrices apply the 4
    bilinear terms and transpose [bin, ch] -> [ch, bin] into fp32 PSUM;
    DVE/Act copies assemble bf16 [256ch, 1024bin]; 2 HWDGE DMAs write the
    ROI's NCHW output (bf16; host converts back to f32).
"""

import numpy as np

B, C, H, W = 8, 256, 128, 128
N_ROIS = 256
PH, PW = 16, 64
NB = PH * PW              # 1024 bins per ROI
NCORES = 8
R = 32                    # ROIs per core
SCALE = 0.25
PIX = H * W               # 16384 pixel rows per batch
TROWS = 2 * PIX           # table rows addressable by int16 idx (32768)
PADROWS = 192             # zero rows appended (idx+1 overrun)
BUFROWS = TROWS + PADROWS

_cache = {}


def _basis_const():
    """M[k, t]: X[n, t] = sum_k ctrl_x[n, k] * M[k, t], t = i*64 + j."""
    t = np.arange(NB)
    i, j = t // PW, t % PW
    u = j.astype(np.float64) / PW
    v = i.astype(np.float64) / PH
    co = [1.0, 3.0, 3.0, 1.0]
    M = np.zeros((8, NB), np.float64)
    for k in range(4):
        bern = co[k] * u**k * (1.0 - u) ** (3 - k)
        M[k] = SCALE * bern * (1.0 - v)       # top curve
        M[4 + k] = SCALE * bern * v           # bottom curve
    # Permute columns to the on-device (g, p) layout. Bin-at-position map
    # t(p, g) = (p%16)*64 + g*8 + p//16 makes DMA engine r (= position%16)
    # walk output row r along the curve -> near-contiguous gather addresses.
    gg = np.arange(8)[:, None]
    pp = np.arange(128)[None, :]
    tcol = (pp % 16) * 64 + gg * 8 + pp // 16
    return M[:, tcol.reshape(-1)].astype(np.float32)


def _build_program(rep=1, variant="full"):
    """rep>1 wraps the main ROI loop in a hardware repeat loop (benchmarking
    only — output is rewritten identically each iteration).
    variant: 'full' | 'gatheronly' | 'nogather' | 'noout' (A/B timing)."""
    do_gather = variant in ("full", "gatheronly", "noout")
    do_compute = variant in ("full", "nogather", "noout")
    do_out = variant in ("full", "nogather")
    import contextlib
    import concourse.bass as bass
    import concourse.bacc as bacc
    import concourse.tile as tile
    from concourse import mybir

    f32 = mybir.dt.float32
    bf16 = mybir.dt.bfloat16
    Alu = mybir.AluOpType
    Act = mybir.ActivationFunctionType

    nc = bacc.Bacc("TRN2", target_bir_lowering=False, debug=False)
    feat = nc.dram_tensor("feat", [BUFROWS, 2 * C], bf16, kind="ExternalInput")
    bez = nc.dram_tensor("bez", [R, 17], f32, kind="ExternalInput")
    mconst = nc.dram_tensor("mconst", [8, NB], f32, kind="ExternalInput")
    eye_d = nc.dram_tensor("eye", [128, 128], f32, kind="ExternalInput")
    eyeb_d = nc.dram_tensor("eyeb", [128, 128], bf16, kind="ExternalInput")
    id32_d = nc.dram_tensor("id32", [32, 32], f32, kind="ExternalInput")
    c16k_d = nc.dram_tensor("c16k", [1, 128], f32, kind="ExternalInput")
    out_d = nc.dram_tensor("out", [R, C, PH, PW], bf16, kind="ExternalOutput")

    # gather table view: idx unit = 1 table row (512 bf16), elem = 2 rows
    gather_src = bass.AP(feat[:].tensor, 0, [[2 * C, TROWS], [1, 4 * C]])

    with tile.TileContext(nc) as tc:
        with (
            tc.tile_pool(name="const", bufs=1) as cpool,
            tc.tile_pool(name="work", bufs=1) as wpool,
        ):
            eye = cpool.tile([128, 128], f32)
            nc.sync.dma_start(eye[:], eye_d[:])
            eyeb = cpool.tile([128, 128], bf16)
            nc.sync.dma_start(eyeb[:], eyeb_d[:])
            m_sb = cpool.tile([8, NB], f32)
            nc.sync.dma_start(m_sb[:], mconst[:])
            id32 = cpool.tile([32, 32], f32)
            nc.sync.dma_start(id32[:], id32_d[:])
            c16k = cpool.tile([1, 128], f32)
            nc.sync.dma_start(c16k[:], c16k_d[:])
            bez_sb = cpool.tile([R, 17], f32)
            nc.sync.dma_start(bez_sb[:], bez[:])

            w00 = wpool.tile([128, 256], f32)
            w01 = wpool.tile([128, 256], f32)
            w10 = wpool.tile([128, 256], f32)
            w11 = wpool.tile([128, 256], f32)
            widx = wpool.tile([128, R * 64], mybir.dt.int16)

            with (
                tc.tile_pool(name="setup", bufs=1) as spool,
                tc.tile_pool(name="psetup", bufs=1, space="PSUM") as pspool,
            ):
                # --- control points -> per-bin coords ------------------
                p_sep = spool.tile([R, 17], f32)
                nc.vector.tensor_copy(p_sep[:, 0:8], bez_sb[:, 1:17:2])
                nc.vector.tensor_copy(p_sep[:, 8:16], bez_sb[:, 2:17:2])
                nc.vector.tensor_copy(p_sep[:, 16:17], bez_sb[:, 0:1])
                pt_ps = pspool.tile([8, 3 * 32], f32)
                nc.tensor.transpose(out=pt_ps[0:8, 0:32], in_=p_sep[:, 0:8],
                                    identity=id32[:])
                nc.tensor.transpose(out=pt_ps[0:8, 32:64], in_=p_sep[:, 8:16],
                                    identity=id32[:])
                nc.tensor.transpose(out=pt_ps[0:1, 64:96], in_=p_sep[:, 16:17],
                                    identity=id32[:])
                pt = spool.tile([8, 3 * 32], f32)
                nc.vector.tensor_copy(pt[:, 0:64], pt_ps[0:8, 0:64])
                nc.vector.tensor_copy(pt[0:1, 64:96], pt_ps[0:1, 64:96])

                ps_x = pspool.tile([128, 256], f32)
                ps_y = pspool.tile([128, 256], f32)
                ps_b = pspool.tile([128, 256], f32)
                for g in range(8):
                    sl = slice(g * 32, (g + 1) * 32)
                    nc.tensor.matmul(out=ps_x[:, sl], lhsT=m_sb[:, g * 128:(g + 1) * 128],
                                     rhs=pt[0:8, 0:32], start=True, stop=True)
                    nc.tensor.matmul(out=ps_y[:, sl], lhsT=m_sb[:, g * 128:(g + 1) * 128],
                                     rhs=pt[0:8, 32:64], start=True, stop=True)
                    nc.tensor.matmul(out=ps_b[:, sl], lhsT=c16k[:],
                                     rhs=pt[0:1, 64:96], start=True, stop=True)

                # --- bilinear indices + weights ------------------------
                def T(name):
                    return spool.tile([128, 256], f32, name=name)

                xs, xl, lx, hx = T("xs"), T("xl"), T("lx"), T("hx")
                ys, yl, ly, hy = T("ys"), T("yl"), T("ly"), T("hy")
                tmp, val, tv = T("tmp"), T("val"), T("tv")
                ixf = T("ixf")

                v = nc.vector
                MAGIC = 12582912.0  # 1.5 * 2**23: (x+M)-M rounds x to nearest
                v.tensor_scalar(xs[:], ps_x[:], 0.0, None, Alu.max)
                v.tensor_scalar(xl[:], xs[:], MAGIC, -MAGIC, Alu.add, Alu.add)
                v.tensor_tensor(tmp[:], xl[:], xs[:], Alu.is_gt)
                v.tensor_tensor(xl[:], xl[:], tmp[:], Alu.subtract)
                v.tensor_scalar(xl[:], xl[:], float(W - 1), None, Alu.min)
                v.tensor_tensor(lx[:], xs[:], xl[:], Alu.subtract)
                v.tensor_scalar(tmp[:], xl[:], float(W - 1), None, Alu.is_lt)
                v.tensor_tensor(lx[:], lx[:], tmp[:], Alu.mult)
                v.tensor_scalar(hx[:], lx[:], 1.0, -1.0, Alu.subtract, Alu.mult)

                v.tensor_scalar(ys[:], ps_y[:], 0.0, None, Alu.max)
                v.tensor_scalar(yl[:], ys[:], MAGIC, -MAGIC, Alu.add, Alu.add)
                v.tensor_tensor(tmp[:], yl[:], ys[:], Alu.is_gt)
                v.tensor_tensor(yl[:], yl[:], tmp[:], Alu.subtract)
                v.tensor_scalar(yl[:], yl[:], float(H - 1), None, Alu.min)
                v.tensor_tensor(ly[:], ys[:], yl[:], Alu.subtract)
                v.tensor_scalar(tmp[:], yl[:], float(H - 1), None, Alu.is_lt)
                v.tensor_tensor(ly[:], ly[:], tmp[:], Alu.mult)
                v.tensor_scalar(hy[:], ly[:], 1.0, -1.0, Alu.subtract, Alu.mult)

                v.tensor_scalar(val[:], ps_x[:], float(W), None, Alu.is_lt)
                v.tensor_scalar(tv[:], ps_x[:], -1.0, None, Alu.is_gt)
                v.tensor_tensor(val[:], val[:], tv[:], Alu.mult)
                v.tensor_scalar(tv[:], ps_y[:], float(H), None, Alu.is_lt)
                v.tensor_tensor(val[:], val[:], tv[:], Alu.mult)
                v.tensor_scalar(tv[:], ps_y[:], -1.0, None, Alu.is_gt)
                v.tensor_tensor(val[:], val[:], tv[:], Alu.mult)
                v.tensor_tensor(hy[:], hy[:], val[:], Alu.mult)
                v.tensor_tensor(ly[:], ly[:], val[:], Alu.mult)

                v.tensor_tensor(w00[:], hy[:], hx[:], Alu.mult)
                v.tensor_tensor(w01[:], hy[:], lx[:], Alu.mult)
                v.tensor_tensor(w10[:], ly[:], hx[:], Alu.mult)
                v.tensor_tensor(w11[:], ly[:], lx[:], Alu.mult)

                v.tensor_scalar(ixf[:], yl[:], float(W), None, Alu.mult)
                v.tensor_tensor(ixf[:], ixf[:], xl[:], Alu.add)
                v.tensor_tensor(ixf[:], ixf[:], ps_b[:], Alu.add)

                # --- fold idx to dma_gather wrapped layout -------------
                # widx[r, n*64 + g*8 + q] = ix[q*16+r, g*32+n]
                fold = pspool.tile([16, 2048], f32, name="fold", tag="fold")
                for q in range(8):
                    nc.tensor.matmul(out=fold[:, q * 256:(q + 1) * 256],
                                     lhsT=eye[:, q * 16:(q + 1) * 16],
                                     rhs=ixf[:], start=True, stop=True)
                dst = (widx[0:16, :]
                       .rearrange("r (n t) -> r n t", n=R)
                       .rearrange("r n (g q) -> r n g q", g=8))
                perm = fold[:].rearrange("r (q g n) -> r n g q", q=8, g=8, n=32)
                nc.vector.tensor_copy(dst, perm)
                for k in range(1, 8):
                    nc.sync.dma_start(widx[16 * k:16 * (k + 1), :], widx[0:16, :])

            # --- main ROI loop ------------------------------------------
            ga_static = None
            if not do_gather:
                ga_static = wpool.tile([128, 8 * 1024], bf16, name="ga_static")
                nc.vector.memset(ga_static[:], 0.0)
            with (
                tc.tile_pool(name="gath", bufs=3) as gpool,
                tc.tile_pool(name="stg", bufs=3) as stpool,
                tc.tile_pool(name="diag", bufs=3) as dpool,
                tc.tile_pool(name="pmain", bufs=8, space="PSUM") as ppool,
                tc.For_i(0, rep, 1) if rep > 1 else contextlib.nullcontext(),
            ):
                for n in range(R):
                    ga = gpool.tile([128, 8 * 1024], bf16, name="ga") \
                        if do_gather else ga_static
                    if do_gather:
                        nc.gpsimd.dma_gather(
                            out_ap=ga[:].rearrange("p (t e) -> p t e", e=1024),
                            in_ap=gather_src,
                            idxs_ap=widx[:, n * 64:(n + 1) * 64],
                            num_idxs=1024,
                            num_idxs_reg=1024,
                            elem_size=1024,
                            elem_step=512,
                        )
                    stage = stpool.tile([128, 2048], bf16, name="stage")
                    for g in range(8 if do_compute else 0):
                        col = g * 32 + n
                        d00 = dpool.tile([128, 128], bf16, name="d00")
                        d01 = dpool.tile([128, 128], bf16, name="d01")
                        d10 = dpool.tile([128, 128], bf16, name="d10")
                        d11 = dpool.tile([128, 128], bf16, name="d11")
                        nc.vector.tensor_scalar(d00[:], eyeb[:], w00[:, col:col + 1],
                                                None, Alu.mult)
                        nc.scalar.activation(d01[:], eyeb[:], Act.Copy,
                                             scale=w01[:, col:col + 1])
                        nc.vector.tensor_scalar(d10[:], eyeb[:], w10[:, col:col + 1],
                                                None, Alu.mult)
                        nc.scalar.activation(d11[:], eyeb[:], Act.Copy,
                                             scale=w11[:, col:col + 1])
                        po = ppool.tile([128, 256], f32, name="po")
                        for h in range(2):
                            osl = slice(h * 128, (h + 1) * 128)
                            base = g * 1024 + h * 128
                            nc.tensor.matmul(out=po[:, osl], rhs=d00[:],
                                             lhsT=ga[:, base:base + 128],
                                             start=True, stop=False)
                            nc.tensor.matmul(out=po[:, osl], rhs=d10[:],
                                             lhsT=ga[:, base + 256:base + 384],
                                             start=False, stop=False)
                            nc.tensor.matmul(out=po[:, osl], rhs=d01[:],
                                             lhsT=ga[:, base + 512:base + 640],
                                             start=False, stop=False)
                            nc.tensor.matmul(out=po[:, osl], rhs=d11[:],
                                             lhsT=ga[:, base + 768:base + 896],
                                             start=False, stop=True)
                        # scatter psum col p -> stage bin t(p,g); iterate
                        # (a = p//16 outer, r = p%16 inner) on both sides
                        for h in range(2):
                            src = (po[:, h * 128:(h + 1) * 128]
                                   .rearrange("c (a r) -> c a r", a=8))
                            dst = (stage[:, h * 1024:(h + 1) * 1024]
                                   .rearrange("c (r g2 a) -> c g2 a r",
                                              r=16, g2=8, a=8)[:, g, :, :])
                            nc.vector.tensor_copy(dst, src)
                    for h in range(2 if do_out else 0):
                        nc.sync.dma_start(out_d[n, h * 128:(h + 1) * 128, :, :],
                                          stage[:, h * 1024:(h + 1) * 1024])

    nc.compile()
    return nc


def _route(batch):
    """Assign ROIs to cores: sorted by batch, each core spans <=2 adjacent
    batches, <=R ROIs. Returns (ids_per_core, base_per_core)."""
    order = np.argsort(batch, kind="stable")
    n = len(order)
    ids, bases = [], []
    i = 0
    for _ in range(NCORES):
        if i >= n:
            ids.append([])
            bases.append(0)
            continue
        base = int(batch[order[i]])
        cur = []
        while i < n and len(cur) < R and int(batch[order[i]]) <= base + 1:
            cur.append(int(order[i]))
            i += 1
        ids.append(cur)
        bases.append(base)
    if i < n:
        raise RuntimeError("ROI->core routing failed (batch distribution too "
                           "skewed for 8 cores x 2 batches)")
    return ids, bases


def kernel(input, beziers):
    import ml_dtypes
    from concourse.bass_utils import run_bass_kernel_spmd

    bf16 = ml_dtypes.bfloat16
    input = np.asarray(input, dtype=np.float32)
    beziers = np.asarray(beziers, dtype=np.float32)

    if "nc" not in _cache:
        _cache["nc"] = _build_program()
    nc = _cache["nc"]

    batch = beziers[:, 0].astype(np.int32)
    ids, bases = _route(batch)

    nhwc = np.ascontiguousarray(
        input.transpose(0, 2, 3, 1)).reshape(B * PIX, C).astype(bf16)
    consts = {
        "mconst": _basis_const(),
        "eye": np.eye(128, dtype=np.float32),
        "eyeb": np.eye(128, dtype=np.float32).astype(bf16),
        "id32": np.eye(32, dtype=np.float32),
        "c16k": np.full((1, 128), float(PIX), np.float32),
    }

    in_maps = []
    for c in range(NCORES):
        buf = np.zeros((BUFROWS, 2 * C), bf16)
        lo = bases[c] * PIX
        hi = min((bases[c] + 2) * PIX, B * PIX)
        nrow = hi - lo
        buf[:nrow, :C] = nhwc[lo:hi]
        # second half of row t = pixel t + W (next image row, same layout)
        buf[:nrow - W, C:] = nhwc[lo + W:hi]
        if hi < B * PIX:
            buf[nrow - W:nrow, C:] = nhwc[hi:hi + W]
        bz = np.zeros((R, 17), np.float32)
        if ids[c]:
            rows = beziers[ids[c]].copy()
            rows[:, 0] = batch[ids[c]] - bases[c]
            bz[:len(ids[c])] = rows
        in_maps.append({"feat": buf, "bez": bz, **consts})

    _cache["in_maps"] = in_maps
    res = run_bass_kernel_spmd(nc, in_maps, list(range(NCORES)))

    out = np.zeros((N_ROIS, C, PH, PW), np.float32)
    for c in range(NCORES):
        if ids[c]:
            out[ids[c]] = res.results[c]["out"][:len(ids[c])].astype(np.float32)
    return out


# revision 10
# speedup vs baseline: 2.1974x; 1.4921x over previous
"""BezierAlign distributed Trainium2 kernel.

Contract: kernel(input, beziers) -> [256, 256, 16, 64] f32, computed on the
8 NeuronCores. Host side only shards/routes/reassembles:
  - ROIs are routed to cores so each core's 32 ROIs live in <= 2 adjacent
    batches (sharding_hint: route ROIs by batch / shard by ROI).
  - The feature map is resharded per core as a bf16 y-paired NHWC pixel
    table (2 batches): table row t = [ch(pixel t), ch(pixel t + W)], so one
    2-row gather element covers a bin's full 2x2 bilinear patch.
All math (bezier eval, bilinear weights, index arithmetic, gather, weighted
sum, output transpose) runs on-device.

Per-core device program (SPMD, identical on all 8 cores):
  - bezier control points -> sample coords X, Y for all 32x1024 bins via
    PE matmuls against a constant basis matrix (layout [bin%128, g*32+n]).
  - bilinear indices/weights via DVE elementwise ops; gather indices are
    folded to dma_gather's 16-partition-wrapped int16 layout with identity
    -slice PE matmuls + one permuted copy.
  - per ROI: one 1024-index dma_gather pulls 2 MiB of bf16 2x2-patch rows;
    8 groups x 8 bf16 matmuls against diag(weight) matrices apply the 4
    bilinear terms and transpose [bin, ch] -> [ch, bin] into fp32 PSUM;
    DVE/Act copies assemble bf16 [256ch, 1024bin]; 2 HWDGE DMAs write the
    ROI's NCHW output (bf16; host converts back to f32).
"""

import numpy as np

B, C, H, W = 8, 256, 128, 128
N_ROIS = 256
PH, PW = 16, 64
NB = PH * PW              # 1024 bins per ROI
NCORES = 8
R = 32                    # ROIs per core
SCALE = 0.25
PIX = H * W               # 16384 pixel rows per batch
TROWS = 2 * PIX           # table rows addressable by int16 idx (32768)
PADROWS = 192             # zero rows appended (idx+1 overrun)
BUFROWS = TROWS + PADROWS

_cache = {}


def _basis_const():
    """M[k, t]: X[n, t] = sum_k ctrl_x[n, k] * M[k, t], t = i*64 + j."""
    t = np.arange(NB)
    i, j = t // PW, t % PW
    u = j.astype(np.float64) / PW
    v = i.astype(np.float64) / PH
    co = [1.0, 3.0, 3.0, 1.0]
    M = np.zeros((8, NB), np.float64)
    for k in range(4):
        bern = co[k] * u**k * (1.0 - u) ** (3 - k)
        M[k] = SCALE * bern * (1.0 - v)       # top curve
        M[4 + k] = SCALE * bern * v           # bottom curve
    # Permute columns to the on-device (g, p) layout. Bin-at-position map
    # t(p, g) = (p%16)*64 + g*8 + p//16 makes DMA engine r (= position%16)
    # walk output row r along the curve -> near-contiguous gather addresses.
    gg = np.arange(8)[:, None]
    pp = np.arange(128)[None, :]
    tcol = (pp % 16) * 64 + gg * 8 + pp // 16
    return M[:, tcol.reshape(-1)].astype(np.float32)


def _build_program(rep=1, variant="full"):
    """rep>1 wraps the main ROI loop in a hardware repeat loop (benchmarking
    only — output is rewritten identically each iteration).
    variant: 'full' | 'gatheronly' | 'gatherzero' | 'nogather' | 'noout'
    (A/B timing; gatherzero = gather-only with all-zero indices to probe
    locality sensitivity of the gather rate)."""
    do_gather = variant in ("full", "gatheronly", "gatherzero", "noout")
    do_compute = variant in ("full", "nogather", "noout")
    do_out = variant in ("full", "nogather")
    import contextlib
    import concourse.bass as bass
    import concourse.bacc as bacc
    import concourse.tile as tile
    from concourse import mybir

    f32 = mybir.dt.float32
    bf16 = mybir.dt.bfloat16
    Alu = mybir.AluOpType
    Act = mybir.ActivationFunctionType

    nc = bacc.Bacc("TRN2", target_bir_lowering=False, debug=False)
    feat = nc.dram_tensor("feat", [BUFROWS, 2 * C], bf16, kind="ExternalInput")
    bez = nc.dram_tensor("bez", [R, 17], f32, kind="ExternalInput")
    mconst = nc.dram_tensor("mconst", [8, NB], f32, kind="ExternalInput")
    eye_d = nc.dram_tensor("eye", [128, 128], f32, kind="ExternalInput")
    eyeb_d = nc.dram_tensor("eyeb", [128, 128], bf16, kind="ExternalInput")
    id32_d = nc.dram_tensor("id32", [32, 32], f32, kind="ExternalInput")
    c16k_d = nc.dram_tensor("c16k", [1, 128], f32, kind="ExternalInput")
    out_d = nc.dram_tensor("out", [R, C, PH, PW], bf16, kind="ExternalOutput")

    # gather table view: idx unit = 1 table row (512 bf16), elem = 2 rows
    gather_src = bass.AP(feat[:].tensor, 0, [[2 * C, TROWS], [1, 4 * C]])

    with tile.TileContext(nc) as tc:
        with (
            tc.tile_pool(name="const", bufs=1) as cpool,
            tc.tile_pool(name="work", bufs=1) as wpool,
        ):
            eye = cpool.tile([128, 128], f32)
            nc.sync.dma_start(eye[:], eye_d[:])
            eyeb = cpool.tile([128, 128], bf16)
            nc.sync.dma_start(eyeb[:], eyeb_d[:])
            m_sb = cpool.tile([8, NB], f32)
            nc.sync.dma_start(m_sb[:], mconst[:])
            id32 = cpool.tile([32, 32], f32)
            nc.sync.dma_start(id32[:], id32_d[:])
            c16k = cpool.tile([1, 128], f32)
            nc.sync.dma_start(c16k[:], c16k_d[:])
            bez_sb = cpool.tile([R, 17], f32)
            nc.sync.dma_start(bez_sb[:], bez[:])

            w00 = wpool.tile([128, 256], f32)
            w01 = wpool.tile([128, 256], f32)
            w10 = wpool.tile([128, 256], f32)
            w11 = wpool.tile([128, 256], f32)
            widx = wpool.tile([128, R * 64], mybir.dt.int16)

            with (
                tc.tile_pool(name="setup", bufs=1) as spool,
                tc.tile_pool(name="psetup", bufs=1, space="PSUM") as pspool,
            ):
                # --- control points -> per-bin coords ------------------
                p_sep = spool.tile([R, 17], f32)
                nc.vector.tensor_copy(p_sep[:, 0:8], bez_sb[:, 1:17:2])
                nc.vector.tensor_copy(p_sep[:, 8:16], bez_sb[:, 2:17:2])
                nc.vector.tensor_copy(p_sep[:, 16:17], bez_sb[:, 0:1])
                pt_ps = pspool.tile([8, 3 * 32], f32)
                nc.tensor.transpose(out=pt_ps[0:8, 0:32], in_=p_sep[:, 0:8],
                                    identity=id32[:])
                nc.tensor.transpose(out=pt_ps[0:8, 32:64], in_=p_sep[:, 8:16],
                                    identity=id32[:])
                nc.tensor.transpose(out=pt_ps[0:1, 64:96], in_=p_sep[:, 16:17],
                                    identity=id32[:])
                pt = spool.tile([8, 3 * 32], f32)
                nc.vector.tensor_copy(pt[:, 0:64], pt_ps[0:8, 0:64])
                nc.vector.tensor_copy(pt[0:1, 64:96], pt_ps[0:1, 64:96])

                ps_x = pspool.tile([128, 256], f32)
                ps_y = pspool.tile([128, 256], f32)
                ps_b = pspool.tile([128, 256], f32)
                for g in range(8):
                    sl = slice(g * 32, (g + 1) * 32)
                    nc.tensor.matmul(out=ps_x[:, sl], lhsT=m_sb[:, g * 128:(g + 1) * 128],
                                     rhs=pt[0:8, 0:32], start=True, stop=True)
                    nc.tensor.matmul(out=ps_y[:, sl], lhsT=m_sb[:, g * 128:(g + 1) * 128],
                                     rhs=pt[0:8, 32:64], start=True, stop=True)
                    nc.tensor.matmul(out=ps_b[:, sl], lhsT=c16k[:],
                                     rhs=pt[0:1, 64:96], start=True, stop=True)

                # --- bilinear indices + weights ------------------------
                def T(name):
                    return spool.tile([128, 256], f32, name=name)

                xs, xl, lx, hx = T("xs"), T("xl"), T("lx"), T("hx")
                ys, yl, ly, hy = T("ys"), T("yl"), T("ly"), T("hy")
                tmp, val, tv = T("tmp"), T("val"), T("tv")
                ixf = T("ixf")

                v = nc.vector
                MAGIC = 12582912.0  # 1.5 * 2**23: (x+M)-M rounds x to nearest
                v.tensor_scalar(xs[:], ps_x[:], 0.0, None, Alu.max)
                v.tensor_scalar(xl[:], xs[:], MAGIC, -MAGIC, Alu.add, Alu.add)
                v.tensor_tensor(tmp[:], xl[:], xs[:], Alu.is_gt)
                v.tensor_tensor(xl[:], xl[:], tmp[:], Alu.subtract)
                v.tensor_scalar(xl[:], xl[:], float(W - 1), None, Alu.min)
                v.tensor_tensor(lx[:], xs[:], xl[:], Alu.subtract)
                v.tensor_scalar(tmp[:], xl[:], float(W - 1), None, Alu.is_lt)
                v.tensor_tensor(lx[:], lx[:], tmp[:], Alu.mult)
                v.tensor_scalar(hx[:], lx[:], 1.0, -1.0, Alu.subtract, Alu.mult)

                v.tensor_scalar(ys[:], ps_y[:], 0.0, None, Alu.max)
                v.tensor_scalar(yl[:], ys[:], MAGIC, -MAGIC, Alu.add, Alu.add)
                v.tensor_tensor(tmp[:], yl[:], ys[:], Alu.is_gt)
                v.tensor_tensor(yl[:], yl[:], tmp[:], Alu.subtract)
                v.tensor_scalar(yl[:], yl[:], float(H - 1), None, Alu.min)
                v.tensor_tensor(ly[:], ys[:], yl[:], Alu.subtract)
                v.tensor_scalar(tmp[:], yl[:], float(H - 1), None, Alu.is_lt)
                v.tensor_tensor(ly[:], ly[:], tmp[:], Alu.mult)
                v.tensor_scalar(hy[:], ly[:], 1.0, -1.0, Alu.subtract, Alu.mult)

                v.tensor_scalar(val[:], ps_x[:], float(W), None, Alu.is_lt)
                v.tensor_scalar(tv[:], ps_x[:], -1.0, None, Alu.is_gt)
                v.tensor_tensor(val[:], val[:], tv[:], Alu.mult)
                v.tensor_scalar(tv[:], ps_y[:], float(H), None, Alu.is_lt)
                v.tensor_tensor(val[:], val[:], tv[:], Alu.mult)
                v.tensor_scalar(tv[:], ps_y[:], -1.0, None, Alu.is_gt)
                v.tensor_tensor(val[:], val[:], tv[:], Alu.mult)
                v.tensor_tensor(hy[:], hy[:], val[:], Alu.mult)
                v.tensor_tensor(ly[:], ly[:], val[:], Alu.mult)

                v.tensor_tensor(w00[:], hy[:], hx[:], Alu.mult)
                v.tensor_tensor(w01[:], hy[:], lx[:], Alu.mult)
                v.tensor_tensor(w10[:], ly[:], hx[:], Alu.mult)
                v.tensor_tensor(w11[:], ly[:], lx[:], Alu.mult)

                v.tensor_scalar(ixf[:], yl[:], float(W), None, Alu.mult)
                v.tensor_tensor(ixf[:], ixf[:], xl[:], Alu.add)
                v.tensor_tensor(ixf[:], ixf[:], ps_b[:], Alu.add)

                # --- fold idx to dma_gather wrapped layout -------------
                # widx[r, n*64 + g*8 + q] = ix[q*16+r, g*32+n]
                fold = pspool.tile([16, 2048], f32, name="fold", tag="fold")
                for q in range(8):
                    nc.tensor.matmul(out=fold[:, q * 256:(q + 1) * 256],
                                     lhsT=eye[:, q * 16:(q + 1) * 16],
                                     rhs=ixf[:], start=True, stop=True)
                dst = (widx[0:16, :]
                       .rearrange("r (n t) -> r n t", n=R)
                       .rearrange("r n (g q) -> r n g q", g=8))
                perm = fold[:].rearrange("r (q g n) -> r n g q", q=8, g=8, n=32)
                nc.vector.tensor_copy(dst, perm)
                for k in range(1, 8):
                    nc.sync.dma_start(widx[16 * k:16 * (k + 1), :], widx[0:16, :])

            # --- main ROI loop (software-pipelined) ---------------------
            # Iteration n: gather(n) + diag(n) + matmuls(n) into PSUM, and
            # PSUM->stage copies + output DMA for ROI n-1. Per-engine streams
            # are in-order, so diag builds for the next ROI must be issued
            # BEFORE the previous ROI's stage copies or the DVE head blocks
            # on PE, serializing the gather against all compute.
            ga_static = None
            if not do_gather:
                ga_static = wpool.tile([128, 8 * 1024], bf16, name="ga_static")
                nc.vector.memset(ga_static[:], 0.0)
            if variant == "gatherzero":
                widx0 = wpool.tile([128, R * 64], mybir.dt.int16)
                nc.vector.memset(widx0[:], 0)
                widx = widx0
            with (
                tc.tile_pool(name="gath", bufs=3) as gpool,
                tc.tile_pool(name="stg", bufs=3) as stpool,
                tc.tile_pool(name="diag", bufs=4) as dpool,
                tc.tile_pool(name="pmain", bufs=8, space="PSUM") as ppool,
                tc.For_i(0, rep, 1) if rep > 1 else contextlib.nullcontext(),
            ):
                po_prev = None
                for n in range(R + 1):
                    if do_gather and n < R:
                        ga = gpool.tile([128, 8 * 1024], bf16, name="ga")
                        nc.gpsimd.dma_gather(
                            out_ap=ga[:].rearrange("p (t e) -> p t e", e=1024),
                            in_ap=gather_src,
                            idxs_ap=widx[:, n * 64:(n + 1) * 64],
                            num_idxs=1024,
                            num_idxs_reg=1024,
                            elem_size=1024,
                            elem_step=512,
                        )
                    elif n < R:
                        ga = ga_static
                    stage = stpool.tile([128, 2048], bf16, name="stage") \
                        if (do_compute and n > 0) else None
                    po_cur = []
                    for g in range(8 if do_compute else 0):
                        if n < R:
                            col = g * 32 + n
                            d00 = dpool.tile([128, 128], bf16, name="d00")
                            d01 = dpool.tile([128, 128], bf16, name="d01")
                            d10 = dpool.tile([128, 128], bf16, name="d10")
                            d11 = dpool.tile([128, 128], bf16, name="d11")
                            nc.vector.tensor_scalar(d00[:], eyeb[:],
                                                    w00[:, col:col + 1],
                                                    None, Alu.mult)
                            nc.scalar.activation(d01[:], eyeb[:], Act.Copy,
                                                 scale=w01[:, col:col + 1])
                            nc.vector.tensor_scalar(d10[:], eyeb[:],
                                                    w10[:, col:col + 1],
                                                    None, Alu.mult)
                            nc.scalar.activation(d11[:], eyeb[:], Act.Copy,
                                                 scale=w11[:, col:col + 1])
                        if n > 0:
                            # scatter psum col p -> stage bin t(p,g) for ROI
                            # n-1; p = a*16 + r maps to t = r*64 + g*8 + a
                            pp = po_prev[g]
                            src = pp[:].rearrange("c (h a r) -> c h a r",
                                                  h=2, a=8)
                            dst = (stage[:]
                                   .rearrange("c (h r g2 a) -> c h g2 a r",
                                              h=2, r=16, g2=8, a=8)[:, :, g])
                            if g % 2 == 0:
                                nc.vector.tensor_copy(dst, src)
                            else:
                                nc.scalar.activation(dst, src, Act.Copy)
                        if n < R:
                            po = ppool.tile([128, 256], f32, name="po")
                            po_cur.append(po)
                            for h in range(2):
                                osl = slice(h * 128, (h + 1) * 128)
                                base = g * 1024 + h * 128
                                nc.tensor.matmul(out=po[:, osl], rhs=d00[:],
                                                 lhsT=ga[:, base:base + 128],
                                                 start=True, stop=False)
                                nc.tensor.matmul(out=po[:, osl], rhs=d10[:],
                                                 lhsT=ga[:, base + 256:base + 384],
                                                 start=False, stop=False)
                                nc.tensor.matmul(out=po[:, osl], rhs=d01[:],
                                                 lhsT=ga[:, base + 512:base + 640],
                                                 start=False, stop=False)
                                nc.tensor.matmul(out=po[:, osl], rhs=d11[:],
                                                 lhsT=ga[:, base + 768:base + 896],
                                                 start=False, stop=True)
                    if do_compute and do_out and n > 0:
                        for h in range(2):
                            nc.sync.dma_start(
                                out_d[n - 1, h * 128:(h + 1) * 128, :, :],
                                stage[:, h * 1024:(h + 1) * 1024])
                    po_prev = po_cur

    nc.compile()
    return nc


def _route(batch):
    """Assign ROIs to cores: sorted by batch, each core spans <=2 adjacent
    batches, <=R ROIs. Returns (ids_per_core, base_per_core)."""
    order = np.argsort(batch, kind="stable")
    n = len(order)
    ids, bases = [], []
    i = 0
    for _ in range(NCORES):
        if i >= n:
            ids.append([])
            bases.append(0)
            continue
        base = int(batch[order[i]])
        cur = []
        while i < n and len(cur) < R and int(batch[order[i]]) <= base + 1:
            cur.append(int(order[i]))
            i += 1
        ids.append(cur)
        bases.append(base)
    if i < n:
        raise RuntimeError("ROI->core routing failed (batch distribution too "
                           "skewed for 8 cores x 2 batches)")
    return ids, bases


def kernel(input, beziers):
    import ml_dtypes
    from concourse.bass_utils import run_bass_kernel_spmd

    bf16 = ml_dtypes.bfloat16
    input = np.asarray(input, dtype=np.float32)
    beziers = np.asarray(beziers, dtype=np.float32)

    if "nc" not in _cache:
        _cache["nc"] = _build_program()
    nc = _cache["nc"]

    batch = beziers[:, 0].astype(np.int32)
    ids, bases = _route(batch)

    nhwc = np.ascontiguousarray(
        input.transpose(0, 2, 3, 1)).reshape(B * PIX, C).astype(bf16)
    consts = {
        "mconst": _basis_const(),
        "eye": np.eye(128, dtype=np.float32),
        "eyeb": np.eye(128, dtype=np.float32).astype(bf16),
        "id32": np.eye(32, dtype=np.float32),
        "c16k": np.full((1, 128), float(PIX), np.float32),
    }

    in_maps = []
    for c in range(NCORES):
        buf = np.zeros((BUFROWS, 2 * C), bf16)
        lo = bases[c] * PIX
        hi = min((bases[c] + 2) * PIX, B * PIX)
        nrow = hi - lo
        buf[:nrow, :C] = nhwc[lo:hi]
        # second half of row t = pixel t + W (next image row, same layout)
        buf[:nrow - W, C:] = nhwc[lo + W:hi]
        if hi < B * PIX:
            buf[nrow - W:nrow, C:] = nhwc[hi:hi + W]
        bz = np.zeros((R, 17), np.float32)
        if ids[c]:
            rows = beziers[ids[c]].copy()
            rows[:, 0] = batch[ids[c]] - bases[c]
            bz[:len(ids[c])] = rows
        in_maps.append({"feat": buf, "bez": bz, **consts})

    _cache["in_maps"] = in_maps
    res = run_bass_kernel_spmd(nc, in_maps, list(range(NCORES)))

    out = np.zeros((N_ROIS, C, PH, PW), np.float32)
    for c in range(NCORES):
        if ids[c]:
            out[ids[c]] = res.results[c]["out"][:len(ids[c])].astype(np.float32)
    return out
